# revision 5
# baseline (speedup 1.0000x reference)
"""Trainium2 Bass kernel for nn_ConvBundle_48146583388363.

Math: out[x,y,b,i,j,o] = s[b, i+x-1, j+y-1] * wsum[x,y,o]
  where s = inputs.sum(channel) (zero-padded at borders) and
  wsum = W.sum(axis=2).

Sharding: data-parallel over batch B=16 across 8 cores (2 batches/core).
W and the small structural constants are replicated.

Per-core layout: flattened per-batch spatial index f = 128*t + p
(p = SBUF partition, t = tile column). The 9 tap shifts f -> f+delta
are done with 0/1 shift-matrix matmuls on the tensor engine (plus a
column-border mask), then each output tile [128 spatial, 128 cout] is a
per-partition tensor_scalar outer product, accumulated into [128, 9216]
slabs and DMA'd out as one multi-MB transfer per (tap, batch).

Note: walrus allows only ONE sync-wait on a Matmult (it rides the
LDWEIGHTS struct), so matmul operands are grouped into single DMAs and
a dummy matmul pre-syncs the shift-matrix DMA lane on PE.
"""

import numpy as np

import concourse.bacc as bacc
import concourse.bass as bass
import concourse.mybir as mybir
from concourse import tile
from concourse.bass_utils import run_bass_kernel_spmd

F32 = mybir.dt.float32

NCORES = 8
B, H, W_, CIN = 16, 96, 96, 64
COUT = 128
BPC = B // NCORES          # batches per core = 2
SP = H * W_                # 9216 spatial positions per batch
TPB = SP // 128            # 72 tiles of 128 spatial positions
NTAP = 9
TAPS = [(x - 1, y - 1) for x in range(3) for y in range(3)]  # tap n = 3x+y


def _build_consts():
    """Structural (input-independent) constants, computed on host."""
    shift_ab = np.zeros((2 * NTAP, 128, 128), np.float32)
    for n, (dx, dy) in enumerate(TAPS):
        d = 96 * dx + dy
        if d == 0:
            continue
        for m in range(128):
            k = m + d
            if 0 <= k < 128:
                shift_ab[n, k, m] = 1.0
            elif d > 0:
                shift_ab[NTAP + n, k - 128, m] = 1.0
            else:
                shift_ab[NTAP + n, k + 128, m] = 1.0
    f = 128 * np.arange(TPB)[None, :] + np.arange(128)[:, None]  # [128, 72]
    masks = np.stack([f % 96 != 0, f % 96 != 95]).astype(np.float32)
    return shift_ab, masks


def _build_nc():
    # Bacc (not raw Bass): its finalize() runs move_matmul_waits_to_ldweights
    # + generate_event_semaphores, which split multi-waits to satisfy the
    # 1-sync-wait-per-instruction hardware constraint.
    nc = bacc.Bacc(None, target_bir_lowering=False)
    x = nc.dram_tensor("x", [BPC, SP, CIN], F32, kind="ExternalInput")
    # wc[0] = all-ones (for the colsum matmul), wc[1+n] = W tap n
    wc = nc.dram_tensor("wc", [1 + NTAP, 128, COUT], F32, kind="ExternalInput")
    ab = nc.dram_tensor("ab", [2 * NTAP, 128, 128], F32, kind="ExternalInput")
    mk = nc.dram_tensor("mk", [2, 128, TPB], F32, kind="ExternalInput")
    y = nc.dram_tensor("y", [NTAP, BPC, SP, COUT], F32, kind="ExternalOutput")

    with tile.TileContext(nc) as tc:
        with (
            tc.tile_pool(name="const", bufs=1) as cpool,
            tc.tile_pool(name="xin", bufs=2) as xpool,
            tc.tile_pool(name="sshift", bufs=4) as spool,
            tc.tile_pool(name="psum_w", bufs=2, space="PSUM") as pwpool,
            tc.tile_pool(name="psum_s", bufs=4, space="PSUM") as pspool,
            tc.tile_pool(name="out", bufs=3) as opool,
        ):
            wc_sb = cpool.tile([128, (1 + NTAP) * COUT], F32, name="wc_sb")
            nc.scalar.dma_start(out=wc_sb[:], in_=wc.rearrange("n k m -> k n m"))
            ab_sb = cpool.tile([128, 2 * NTAP * 128], F32, name="ab_sb")
            nc.scalar.dma_start(out=ab_sb[:], in_=ab.rearrange("n k m -> k n m"))
            mk_sb = cpool.tile([128, 2 * TPB], F32, name="mk_sb")
            nc.scalar.dma_start(out=mk_sb[:], in_=mk.rearrange("n p t -> p n t"))

            # wsum[n] = colsum of W[n], replicated across all 128 partitions
            # via ones.T @ W (one matmul does reduce + broadcast).
            ones_ap = wc_sb[:, 0:COUT]
            wsum = []
            for n in range(NTAP):
                pw = pwpool.tile([128, COUT], F32, name=f"pw{n}", tag="pw")
                nc.tensor.matmul(
                    pw[:], lhsT=ones_ap,
                    rhs=wc_sb[:, (1 + n) * COUT:(2 + n) * COUT],
                    start=True, stop=True,
                )
                ws = cpool.tile([128, COUT], F32, name=f"wsum{n}")
                nc.scalar.copy(ws[:], pw[:])
                wsum.append(ws)

            # Dummy matmul: syncs PE against the ab DMA lane so the real
            # shift matmuls carry only the DVE (s_ext) wait.
            junk = pwpool.tile([1, 1], F32, name="junk", tag="junk")
            nc.tensor.matmul(
                junk[:], lhsT=ab_sb[:, 0:1], rhs=ab_sb[:, 0:1],
                start=True, stop=True,
            )

            # s_ext[b][:, 1+t] = s for tile t; cols 0 and TPB+1 stay zero so
            # the neighbor-tile matmul can read past either end.
            s_ext = []
            for b in range(BPC):
                xt = xpool.tile([128, TPB * CIN], F32, name=f"xt{b}", tag="xt")
                nc.scalar.dma_start(
                    out=xt[:], in_=x[b].rearrange("(t p) c -> p t c", p=128)
                )
                se = cpool.tile([128, TPB + 2], F32, name=f"s_ext{b}")
                nc.vector.memset(se[:], 0.0)
                nc.vector.reduce_sum(
                    out=se[:, 1:TPB + 1],
                    in_=xt[:].rearrange("p (t c) -> p t c", c=CIN),
                    axis=mybir.AxisListType.X,
                )
                s_ext.append(se)

            for n, (dx, dy) in enumerate(TAPS):
                d = 96 * dx + dy
                for b in range(BPC):
                    se = s_ext[b]
                    if d == 0:
                        ssh, off = se, 1
                    else:
                        ps = pspool.tile([128, TPB], F32, name=f"ps{n}_{b}", tag="ps")
                        nc.tensor.matmul(
                            ps[:], lhsT=ab_sb[:, n * 128:(n + 1) * 128],
                            rhs=se[:, 1:TPB + 1], start=True, stop=False,
                        )
                        rhs2 = se[:, 2:TPB + 2] if d > 0 else se[:, 0:TPB]
                        nc.tensor.matmul(
                            ps[:], lhsT=ab_sb[:, (NTAP + n) * 128:(NTAP + n + 1) * 128],
                            rhs=rhs2, start=False, stop=True,
                        )
                        st = spool.tile([128, TPB], F32, name=f"ssh{n}_{b}", tag="ssh")
                        if dy != 0:
                            mc = 0 if dy == -1 else 1
                            nc.vector.tensor_mul(
                                st[:], ps[:], mk_sb[:, mc * TPB:(mc + 1) * TPB]
                            )
                        else:
                            nc.vector.tensor_copy(st[:], ps[:])
                        ssh, off = st, 0

                    slab = opool.tile([128, SP], F32, name=f"slab{n}_{b}", tag="slab")
                    for t in range(TPB):
                        dst = slab[:, t * COUT:(t + 1) * COUT]
                        sc = ssh[:, off + t:off + t + 1]
                        if t % 3 == 2:
                            nc.scalar.mul(dst, wsum[n][:], sc)
                        else:
                            nc.vector.tensor_scalar_mul(dst, wsum[n][:], sc)
                    nc.sync.dma_start(
                        out=y[n, b].rearrange("(t p) o -> p t o", p=128),
                        in_=slab[:],
                    )
    nc.finalize()
    return nc


_CACHE = {}


def _get_nc():
    if "nc" not in _CACHE:
        _CACHE["nc"] = _build_nc()
        _CACHE["consts"] = _build_consts()
    return _CACHE["nc"], _CACHE["consts"]


def _run(x_full, w_full, **kwargs):
    nc, (shift_ab, masks) = _get_nc()
    wc = np.concatenate(
        [np.ones((1, 128, COUT), np.float32), w_full.reshape(NTAP, 128, COUT)]
    )
    xr = x_full.reshape(NCORES, BPC, SP, CIN)
    in_maps = [
        {
            "x": np.ascontiguousarray(xr[c]),
            "wc": wc,
            "ab": shift_ab,
            "mk": masks,
        }
        for c in range(NCORES)
    ]
    return run_bass_kernel_spmd(nc, in_maps, core_ids=list(range(NCORES)), **kwargs)


def kernel(**inputs):
    x_full = np.ascontiguousarray(np.asarray(inputs["inputs"], dtype=np.float32))
    w_full = np.ascontiguousarray(np.asarray(inputs["W"], dtype=np.float32))
    res = _run(x_full, w_full)
    ys = [r["y"].reshape(3, 3, BPC, H, W_, COUT) for r in res.results]
    return np.concatenate(ys, axis=2)


# revision 8
# speedup vs baseline: 1.1675x; 1.1675x over previous
"""Trainium2 Bass kernel for nn_ConvBundle_48146583388363.

Math: out[x,y,b,i,j,o] = s[b, i+x-1, j+y-1] * wsum[x,y,o]
  where s = inputs.sum(channel) (zero-padded at borders) and
  wsum = W.sum(axis=2).

Sharding: data-parallel over batch B=16 across 8 cores (2 batches/core).
W and the small structural constants are replicated.

Per-core layout: flattened per-batch spatial index f = 128*t + p
(p = SBUF partition, t = tile column). The 9 tap shifts f -> f+delta
are done with 0/1 shift-matrix matmuls on the tensor engine (plus a
column-border mask), then each output tile [128 spatial, 128 cout] is a
per-partition tensor_scalar outer product, accumulated into [128, 9216]
slabs and DMA'd out as one multi-MB transfer per (tap, batch).

Note: walrus allows only ONE sync-wait on a Matmult (it rides the
LDWEIGHTS struct), so matmul operands are grouped into single DMAs and
a dummy matmul pre-syncs the shift-matrix DMA lane on PE.
"""

import numpy as np

import concourse.bacc as bacc
import concourse.bass as bass
import concourse.mybir as mybir
from concourse import tile
from concourse.bass_utils import run_bass_kernel_spmd

F32 = mybir.dt.float32

NCORES = 8
B, H, W_, CIN = 16, 96, 96, 64
COUT = 128
BPC = B // NCORES          # batches per core = 2
SP = H * W_                # 9216 spatial positions per batch
TPB = SP // 128            # 72 tiles of 128 spatial positions
NTAP = 9
TAPS = [(x - 1, y - 1) for x in range(3) for y in range(3)]  # tap n = 3x+y


def _build_consts():
    """Structural (input-independent) constants, computed on host."""
    shift_ab = np.zeros((2 * NTAP, 128, 128), np.float32)
    for n, (dx, dy) in enumerate(TAPS):
        d = 96 * dx + dy
        if d == 0:
            continue
        for m in range(128):
            k = m + d
            if 0 <= k < 128:
                shift_ab[n, k, m] = 1.0
            elif d > 0:
                shift_ab[NTAP + n, k - 128, m] = 1.0
            else:
                shift_ab[NTAP + n, k + 128, m] = 1.0
    f = 128 * np.arange(TPB)[None, :] + np.arange(128)[:, None]  # [128, 72]
    masks = np.stack([f % 96 != 0, f % 96 != 95]).astype(np.float32)
    return shift_ab, masks


def _build_nc():
    # Bacc (not raw Bass): its finalize() runs move_matmul_waits_to_ldweights
    # + generate_event_semaphores, which split multi-waits to satisfy the
    # 1-sync-wait-per-instruction hardware constraint.
    nc = bacc.Bacc(None, target_bir_lowering=False)
    x = nc.dram_tensor("x", [BPC, SP, CIN], F32, kind="ExternalInput")
    # wc[0] = all-ones (for the colsum matmul), wc[1+n] = W tap n
    wc = nc.dram_tensor("wc", [1 + NTAP, 128, COUT], F32, kind="ExternalInput")
    ab = nc.dram_tensor("ab", [2 * NTAP, 128, 128], F32, kind="ExternalInput")
    mk = nc.dram_tensor("mk", [2, 128, TPB], F32, kind="ExternalInput")
    # y is stored (p, t, o) per (tap, batch): partition-major, so each
    # partition's 72*128 floats are one contiguous 36.9KB DRAM run and the
    # slab DMA is fully linear. Host unshard permutes (p,t)->(t,p).
    y = nc.dram_tensor("y", [NTAP, BPC, 128, TPB * COUT], F32, kind="ExternalOutput")

    with tile.TileContext(nc) as tc:
        with (
            tc.tile_pool(name="const", bufs=1) as cpool,
            tc.tile_pool(name="xin", bufs=2) as xpool,
            tc.tile_pool(name="sshift", bufs=4) as spool,
            tc.tile_pool(name="psum_w", bufs=2, space="PSUM") as pwpool,
            tc.tile_pool(name="psum_s", bufs=4, space="PSUM") as pspool,
            tc.tile_pool(name="out", bufs=3) as opool,
        ):
            wc_sb = cpool.tile([128, (1 + NTAP) * COUT], F32, name="wc_sb")
            nc.scalar.dma_start(out=wc_sb[:], in_=wc.rearrange("n k m -> k n m"))
            ab_sb = cpool.tile([128, 2 * NTAP * 128], F32, name="ab_sb")
            nc.scalar.dma_start(out=ab_sb[:], in_=ab.rearrange("n k m -> k n m"))
            mk_sb = cpool.tile([128, 2 * TPB], F32, name="mk_sb")
            nc.scalar.dma_start(out=mk_sb[:], in_=mk.rearrange("n p t -> p n t"))

            # wsum[n] = colsum of W[n], replicated across all 128 partitions
            # via ones.T @ W (one matmul does reduce + broadcast).
            ones_ap = wc_sb[:, 0:COUT]
            wsum = []
            for n in range(NTAP):
                pw = pwpool.tile([128, COUT], F32, name=f"pw{n}", tag="pw")
                nc.tensor.matmul(
                    pw[:], lhsT=ones_ap,
                    rhs=wc_sb[:, (1 + n) * COUT:(2 + n) * COUT],
                    start=True, stop=True,
                )
                ws = cpool.tile([128, COUT], F32, name=f"wsum{n}")
                nc.scalar.copy(ws[:], pw[:])
                wsum.append(ws)

            # Dummy matmul: syncs PE against the ab DMA lane so the real
            # shift matmuls carry only the DVE (s_ext) wait.
            junk = pwpool.tile([1, 1], F32, name="junk", tag="junk")
            nc.tensor.matmul(
                junk[:], lhsT=ab_sb[:, 0:1], rhs=ab_sb[:, 0:1],
                start=True, stop=True,
            )

            # s_ext[b][:, 1+t] = s for tile t; cols 0 and TPB+1 stay zero so
            # the neighbor-tile matmul can read past either end.
            s_ext = []
            for b in range(BPC):
                xt = xpool.tile([128, TPB * CIN], F32, name=f"xt{b}", tag="xt")
                nc.scalar.dma_start(
                    out=xt[:], in_=x[b].rearrange("(t p) c -> p t c", p=128)
                )
                se = cpool.tile([128, TPB + 2], F32, name=f"s_ext{b}")
                nc.vector.memset(se[:], 0.0)
                nc.vector.reduce_sum(
                    out=se[:, 1:TPB + 1],
                    in_=xt[:].rearrange("p (t c) -> p t c", c=CIN),
                    axis=mybir.AxisListType.X,
                )
                s_ext.append(se)

            for n, (dx, dy) in enumerate(TAPS):
                d = 96 * dx + dy
                for b in range(BPC):
                    se = s_ext[b]
                    if d == 0:
                        ssh, off = se, 1
                    else:
                        ps = pspool.tile([128, TPB], F32, name=f"ps{n}_{b}", tag="ps")
                        nc.tensor.matmul(
                            ps[:], lhsT=ab_sb[:, n * 128:(n + 1) * 128],
                            rhs=se[:, 1:TPB + 1], start=True, stop=False,
                        )
                        rhs2 = se[:, 2:TPB + 2] if d > 0 else se[:, 0:TPB]
                        nc.tensor.matmul(
                            ps[:], lhsT=ab_sb[:, (NTAP + n) * 128:(NTAP + n + 1) * 128],
                            rhs=rhs2, start=False, stop=True,
                        )
                        st = spool.tile([128, TPB], F32, name=f"ssh{n}_{b}", tag="ssh")
                        if dy != 0:
                            mc = 0 if dy == -1 else 1
                            nc.vector.tensor_mul(
                                st[:], ps[:], mk_sb[:, mc * TPB:(mc + 1) * TPB]
                            )
                        else:
                            nc.vector.tensor_copy(st[:], ps[:])
                        ssh, off = st, 0

                    slab = opool.tile([128, SP], F32, name=f"slab{n}_{b}", tag="slab")
                    for t in range(TPB):
                        dst = slab[:, t * COUT:(t + 1) * COUT]
                        sc = ssh[:, off + t:off + t + 1]
                        if t % 3 == 2:
                            nc.scalar.mul(dst, wsum[n][:], sc)
                        else:
                            nc.vector.tensor_scalar_mul(dst, wsum[n][:], sc)
                    nc.sync.dma_start(out=y[n, b], in_=slab[:])
    nc.finalize()
    return nc


_CACHE = {}


def _get_nc():
    if "nc" not in _CACHE:
        _CACHE["nc"] = _build_nc()
        _CACHE["consts"] = _build_consts()
    return _CACHE["nc"], _CACHE["consts"]


def _run(x_full, w_full, **kwargs):
    nc, (shift_ab, masks) = _get_nc()
    wc = np.concatenate(
        [np.ones((1, 128, COUT), np.float32), w_full.reshape(NTAP, 128, COUT)]
    )
    xr = x_full.reshape(NCORES, BPC, SP, CIN)
    in_maps = [
        {
            "x": np.ascontiguousarray(xr[c]),
            "wc": wc,
            "ab": shift_ab,
            "mk": masks,
        }
        for c in range(NCORES)
    ]
    return run_bass_kernel_spmd(nc, in_maps, core_ids=list(range(NCORES)), **kwargs)


def _unshard(results):
    """Per-core y is [9, BPC, 128(p), 72(t)*128(o)]; spatial index is
    f = 128*t + p, so permute (p,t)->(t,p) while gathering."""
    out = np.empty((3, 3, B, H, W_, COUT), np.float32)
    ov = out.reshape(NTAP, B, TPB, 128, COUT)
    for c, r in enumerate(results):
        yc = r["y"].reshape(NTAP, BPC, 128, TPB, COUT)
        ov[:, BPC * c:BPC * (c + 1)] = yc.transpose(0, 1, 3, 2, 4)
    return out


def kernel(**inputs):
    x_full = np.ascontiguousarray(np.asarray(inputs["inputs"], dtype=np.float32))
    w_full = np.ascontiguousarray(np.asarray(inputs["W"], dtype=np.float32))
    res = _run(x_full, w_full)
    return _unshard(res.results)


# revision 12
# speedup vs baseline: 1.3519x; 1.1580x over previous
"""Trainium2 Bass kernel for nn_ConvBundle_48146583388363.

Math: out[x,y,b,i,j,o] = s[b, i+x-1, j+y-1] * wsum[x,y,o]
  where s = inputs.sum(channel) (zero-padded at borders) and
  wsum = W.sum(axis=2).

Sharding: data-parallel over batch B=16 across 8 cores (2 batches/core).
W and the small structural constants are replicated.

Per-core layout: flattened per-batch spatial index f = 128*t + p
(p = SBUF partition, t = tile column). The 9 tap shifts f -> f+delta
are done with 0/1 shift-matrix matmuls on the tensor engine (plus a
column-border mask), then each output tile [128 spatial, 128 cout] is a
per-partition tensor_scalar outer product, accumulated into [128, 9216]
slabs and DMA'd out as one multi-MB transfer per (tap, batch).

Note: walrus allows only ONE sync-wait on a Matmult (it rides the
LDWEIGHTS struct), so matmul operands are grouped into single DMAs and
a dummy matmul pre-syncs the shift-matrix DMA lane on PE.
"""

import numpy as np

import concourse.bacc as bacc
import concourse.bass as bass
import concourse.mybir as mybir
from concourse import tile
from concourse.bass_utils import run_bass_kernel_spmd

F32 = mybir.dt.float32

NCORES = 8
B, H, W_, CIN = 16, 96, 96, 64
COUT = 128
BPC = B // NCORES          # batches per core = 2
SP = H * W_                # 9216 spatial positions per batch
TPB = SP // 128            # 72 tiles of 128 spatial positions
NTAP = 9
TAPS = [(x - 1, y - 1) for x in range(3) for y in range(3)]  # tap n = 3x+y


def _build_consts():
    """Structural (input-independent) constants, computed on host."""
    shift_ab = np.zeros((2 * NTAP, 128, 128), np.float32)
    for n, (dx, dy) in enumerate(TAPS):
        d = 96 * dx + dy
        if d == 0:
            continue
        for m in range(128):
            k = m + d
            if 0 <= k < 128:
                shift_ab[n, k, m] = 1.0
            elif d > 0:
                shift_ab[NTAP + n, k - 128, m] = 1.0
            else:
                shift_ab[NTAP + n, k + 128, m] = 1.0
    f = 128 * np.arange(TPB)[None, :] + np.arange(128)[:, None]  # [128, 72]
    masks = np.stack([f % 96 != 0, f % 96 != 95]).astype(np.float32)
    return shift_ab, masks


def _build_nc():
    # Bacc (not raw Bass): its finalize() runs move_matmul_waits_to_ldweights
    # + generate_event_semaphores, which split multi-waits to satisfy the
    # 1-sync-wait-per-instruction hardware constraint.
    nc = bacc.Bacc(None, target_bir_lowering=False)
    x = nc.dram_tensor("x", [BPC, SP, CIN], F32, kind="ExternalInput")
    # wc[0] = all-ones (for the colsum matmul), wc[1+n] = W tap n
    wc = nc.dram_tensor("wc", [1 + NTAP, 128, COUT], F32, kind="ExternalInput")
    ab = nc.dram_tensor("ab", [2 * NTAP, 128, 128], F32, kind="ExternalInput")
    mk = nc.dram_tensor("mk", [2, 128, TPB], F32, kind="ExternalInput")
    # y is stored (p, t, o) per (tap, batch): partition-major, so each
    # partition's 72*128 floats are one contiguous 36.9KB DRAM run and the
    # slab DMA is fully linear. Host unshard permutes (p,t)->(t,p).
    y = nc.dram_tensor("y", [NTAP, BPC, 128, TPB * COUT], F32, kind="ExternalOutput")

    with tile.TileContext(nc) as tc:
        with (
            tc.tile_pool(name="const", bufs=1) as cpool,
            tc.tile_pool(name="xin", bufs=2) as xpool,
            tc.tile_pool(name="sshift", bufs=4) as spool,
            tc.tile_pool(name="psum_w", bufs=2, space="PSUM") as pwpool,
            tc.tile_pool(name="psum_s", bufs=4, space="PSUM") as pspool,
            tc.tile_pool(name="out", bufs=6) as opool,
        ):
            # Batch loads first on the ACT HWDGE ring (critical path to the
            # first slab); consts go on the otherwise-idle SP ring.
            xts = []
            for b in range(BPC):
                xt = xpool.tile([128, TPB * CIN], F32, name=f"xt{b}", tag="xt")
                nc.scalar.dma_start(
                    out=xt[:], in_=x[b].rearrange("(t p) c -> p t c", p=128)
                )
                xts.append(xt)

            wc_sb = cpool.tile([128, (1 + NTAP) * COUT], F32, name="wc_sb")
            nc.sync.dma_start(out=wc_sb[:], in_=wc.rearrange("n k m -> k n m"))
            ab_sb = cpool.tile([128, 2 * NTAP * 128], F32, name="ab_sb")
            nc.sync.dma_start(out=ab_sb[:], in_=ab.rearrange("n k m -> k n m"))
            mk_sb = cpool.tile([128, 2 * TPB], F32, name="mk_sb")
            nc.sync.dma_start(out=mk_sb[:], in_=mk.rearrange("n p t -> p n t"))

            # wsum[n] = colsum of W[n], replicated across all 128 partitions
            # via ones.T @ W (one matmul does reduce + broadcast).
            ones_ap = wc_sb[:, 0:COUT]
            wsum = []
            for n in range(NTAP):
                pw = pwpool.tile([128, COUT], F32, name=f"pw{n}", tag="pw")
                nc.tensor.matmul(
                    pw[:], lhsT=ones_ap,
                    rhs=wc_sb[:, (1 + n) * COUT:(2 + n) * COUT],
                    start=True, stop=True,
                )
                ws = cpool.tile([128, COUT], F32, name=f"wsum{n}")
                nc.scalar.copy(ws[:], pw[:])
                wsum.append(ws)

            # Dummy matmul: syncs PE against the ab DMA lane so the real
            # shift matmuls carry only the DVE (s_ext) wait.
            junk = pwpool.tile([1, 1], F32, name="junk", tag="junk")
            nc.tensor.matmul(
                junk[:], lhsT=ab_sb[:, 0:1], rhs=ab_sb[:, 0:1],
                start=True, stop=True,
            )

            # s_ext[b][:, 1+t] = s for tile t; cols 0 and TPB+1 stay zero so
            # the neighbor-tile matmul can read past either end. Reduce in
            # halves so the center tap can start on the first half sooner.
            s_ext = []
            hw = TPB // 2
            for b in range(BPC):
                xt = xts[b]
                se = cpool.tile([128, TPB + 2], F32, name=f"s_ext{b}")
                nc.vector.memset(se[:], 0.0)
                xv = xt[:].rearrange("p (t c) -> p t c", c=CIN)
                nc.vector.reduce_sum(
                    out=se[:, 1:1 + hw], in_=xv[:, :hw], axis=mybir.AxisListType.X
                )
                nc.vector.reduce_sum(
                    out=se[:, 1 + hw:1 + TPB], in_=xv[:, hw:], axis=mybir.AxisListType.X
                )
                s_ext.append(se)

            # Center tap first: it depends only on the reduce, not on the
            # shift matmuls, so output DMA starts earliest.
            for n, (dx, dy) in sorted(enumerate(TAPS), key=lambda e: e[1] != (0, 0)):
                d = 96 * dx + dy
                for b in range(BPC):
                    se = s_ext[b]
                    if d == 0:
                        ssh, off = se, 1
                    else:
                        ps = pspool.tile([128, TPB], F32, name=f"ps{n}_{b}", tag="ps")
                        nc.tensor.matmul(
                            ps[:], lhsT=ab_sb[:, n * 128:(n + 1) * 128],
                            rhs=se[:, 1:TPB + 1], start=True, stop=False,
                        )
                        rhs2 = se[:, 2:TPB + 2] if d > 0 else se[:, 0:TPB]
                        nc.tensor.matmul(
                            ps[:], lhsT=ab_sb[:, (NTAP + n) * 128:(NTAP + n + 1) * 128],
                            rhs=rhs2, start=False, stop=True,
                        )
                        st = spool.tile([128, TPB], F32, name=f"ssh{n}_{b}", tag="ssh")
                        if dy != 0:
                            mc = 0 if dy == -1 else 1
                            nc.vector.tensor_mul(
                                st[:], ps[:], mk_sb[:, mc * TPB:(mc + 1) * TPB]
                            )
                        else:
                            nc.vector.tensor_copy(st[:], ps[:])
                        ssh, off = st, 0

                    for h in range(2):
                        t0, t1 = h * (TPB // 2), (h + 1) * (TPB // 2)
                        slab = opool.tile(
                            [128, (TPB // 2) * COUT], F32,
                            name=f"slab{n}_{b}_{h}", tag="slab",
                        )
                        for t in range(t0, t1):
                            dst = slab[:, (t - t0) * COUT:(t - t0 + 1) * COUT]
                            sc = ssh[:, off + t:off + t + 1]
                            if t % 3 == 2:
                                nc.scalar.mul(dst, wsum[n][:], sc)
                            else:
                                nc.vector.tensor_scalar_mul(dst, wsum[n][:], sc)
                        nc.sync.dma_start(
                            out=y[n, b][:, t0 * COUT:t1 * COUT], in_=slab[:]
                        )
    nc.finalize()
    return nc


_CACHE = {}


def _get_nc():
    if "nc" not in _CACHE:
        _CACHE["nc"] = _build_nc()
        _CACHE["consts"] = _build_consts()
    return _CACHE["nc"], _CACHE["consts"]


def _run(x_full, w_full, **kwargs):
    nc, (shift_ab, masks) = _get_nc()
    wc = np.concatenate(
        [np.ones((1, 128, COUT), np.float32), w_full.reshape(NTAP, 128, COUT)]
    )
    xr = x_full.reshape(NCORES, BPC, SP, CIN)
    in_maps = [
        {
            "x": np.ascontiguousarray(xr[c]),
            "wc": wc,
            "ab": shift_ab,
            "mk": masks,
        }
        for c in range(NCORES)
    ]
    return run_bass_kernel_spmd(nc, in_maps, core_ids=list(range(NCORES)), **kwargs)


def _unshard(results):
    """Per-core y is [9, BPC, 128(p), 72(t)*128(o)]; spatial index is
    f = 128*t + p, so permute (p,t)->(t,p) while gathering."""
    out = np.empty((3, 3, B, H, W_, COUT), np.float32)
    ov = out.reshape(NTAP, B, TPB, 128, COUT)
    for c, r in enumerate(results):
        yc = r["y"].reshape(NTAP, BPC, 128, TPB, COUT)
        ov[:, BPC * c:BPC * (c + 1)] = yc.transpose(0, 1, 3, 2, 4)
    return out


def kernel(**inputs):
    x_full = np.ascontiguousarray(np.asarray(inputs["inputs"], dtype=np.float32))
    w_full = np.ascontiguousarray(np.asarray(inputs["W"], dtype=np.float32))
    res = _run(x_full, w_full)
    return _unshard(res.results)
